# revision 7
# baseline (speedup 1.0000x reference)
"""Trainium2 Bass kernel for nn_HadamardProj.

Math:
    out = -scale * (x / (||x||_2 + 1e-8)) @ proj.T + bias
    proj[o, i] = (-1)^popcount(o & i)  for o < 10000, i < 2048.

Since i < 2^11, only the low 11 bits of o matter: proj[o, :] = H2048[o mod 2048, :]
where H2048 is the (symmetric) 2048-point Sylvester Hadamard matrix.  So the
[B,2048]x[2048,10000] projection collapses to a 2048-point transform
y = x @ H2048 plus column replication (10000 = 4*2048 + 1808):

    out[b, m*2048 + j] = r[b] * y[b, j] + bias[m*2048 + j],
    r = 1 / (||x_b|| + eps)        (-scale is baked into the LUT)

Kronecker split H2048 = H4 (x) H512 with i = c*512+p, j = jc*512+jp:

  1. WHT over the 4 input chunks on the VECTOR engine (2 butterfly stages,
     f32 -> bf16):  u_jc[b, p] = sum_c H4[c, jc] * x[b, c*512+p]
  2. PE transposes (bf16, 1 cyc/row) of the 16 u 128-chunks.
  3. 16 bf16 matmuls/tile (vs 64 fp32r in the H16xH128 scheme): for group jc,
     accumulate over pc:  y[b, jc*512+jp] += uT.T @ (-scale * H4[pc,q] H128)
     The rhs sign-pattern LUT has 4 entries (rows of H4), exactly +-scale in
     bf16, so the H512 block structure runs entirely on the tensor engine.
     bf16 stationaries get FWL weight loads that overlap in-flight matmuls
     (the fp32r path serialized 1280 LDWEIGHTS = 256us of PE time).
  4. Combine/replicate r*y+bias split across engines: replicas 0-2 as DVE
     scalar_tensor_tensor straight from PSUM; ACT evacuates ys = r*y to SBUF
     so GPSIMD tensor_add handles replicas 3-4 (tensor_tensor-class DVE ops
     never grab the shared DVE/GpSimd SBUF port pair, so no contention).

Precision: x and the WHT run through bf16 (three 2^-9-relative roundings);
the resulting max relative output error is ~2e-3 against a 2e-2 gate.

Sharding: data-parallel, 2048 batch rows per core across 8 cores. proj is
never read (regenerated as the 4-entry sign-pattern LUT host-side).
"""

import os
import sys

sys.path.insert(0, "/opt/trn_rl_repo")

import numpy as np

B_FULL = 16384
IN = 2048
OUT = 10000
N_CORES = 8
P = 128
B_CORE = B_FULL // N_CORES          # 2048 rows per core
NT = B_CORE // P                    # 16 row tiles per core
EPS = 1e-8
N_REP = 5                           # 4 full 2048 replicas + 1808 tail

# A/B flags (default = fastest known config)
GP_COMBINE = os.environ.get("HAD_GP", "1") == "1"      # replicas 3,4 on GPSIMD
BCAST_BIAS = os.environ.get("HAD_BCAST", "1") == "1"   # bias row + on-chip bcast

_CACHE = {}


def _popcount_parity(a):
    pc = np.zeros_like(a)
    n = int(a.max()).bit_length() if a.size else 1
    for k in range(max(n, 1)):
        pc += (a >> k) & 1
    return pc & 1


def _hadamard(n):
    i = np.arange(n, dtype=np.int64)
    return (1.0 - 2.0 * _popcount_parity(i[:, None] & i[None, :])).astype(np.float32)


def make_lut(scale_val):
    """4 sign-pattern rhs tensors [4, 128, 512]: row pc of H4 (x) H128, times -scale.

    lut[pc][:, q*128:(q+1)*128] = -scale * (-1)^popcount(pc & q) * H128
    """
    H128 = _hadamard(P)
    H4 = _hadamard(4)
    lut = np.empty((4, P, 512), dtype=np.float32)
    for pc in range(4):
        lut[pc] = np.concatenate([H4[pc, q] * H128 for q in range(4)], axis=1)
    return -scale_val * lut


def build_module(nb):
    """Build the per-core Bass module processing nb 128-row tiles."""
    import concourse.bass as bass  # noqa: F401
    from concourse import bacc
    import concourse.mybir as mybir
    import concourse.tile as tile

    f32 = mybir.dt.float32
    bf16 = mybir.dt.bfloat16
    AF = mybir.ActivationFunctionType
    ALU = mybir.AluOpType

    nc = bacc.Bacc("TRN2", target_bir_lowering=False, debug=False)
    x_d = nc.dram_tensor("x", [nb * P, IN], f32, kind="ExternalInput")
    lut_d = nc.dram_tensor("lutb", [4, P, 512], bf16, kind="ExternalInput")
    ident_d = nc.dram_tensor("identb", [P, P], bf16, kind="ExternalInput")
    if BCAST_BIAS:
        bias_d = nc.dram_tensor("bias1", [1, OUT], f32, kind="ExternalInput")
    else:
        bias_d = nc.dram_tensor("biasr", [P, OUT], f32, kind="ExternalInput")
    out_d = nc.dram_tensor("out", [nb * P, OUT], f32, kind="ExternalOutput")

    with tile.TileContext(nc) as tc:
        with (
            tc.tile_pool(name="const", bufs=1) as cp,
            tc.tile_pool(name="xin", bufs=6) as xp,
            tc.tile_pool(name="sq", bufs=2) as sqp,
            tc.tile_pool(name="wht", bufs=2) as wp,
            tc.tile_pool(name="ut", bufs=2) as utp,
            tc.tile_pool(name="ysb", bufs=2) as ysp,
            tc.tile_pool(name="nrm", bufs=8) as nrmp,
            tc.tile_pool(name="stage", bufs=7) as stp,
            tc.tile_pool(name="pt", bufs=2, space="PSUM") as ptp,
            tc.tile_pool(name="yp", bufs=3, space="PSUM") as ypp,
        ):
            identb = cp.tile([P, P], bf16, tag="identb")
            nc.sync.dma_start(identb[:], ident_d[:, :])
            lutb = cp.tile([P, 4, 512], bf16, tag="lutb")
            nc.sync.dma_start(lutb[:], lut_d[:, :, :].rearrange("g p n -> p g n"))
            biasr = cp.tile([P, OUT], f32, tag="biasr")
            if BCAST_BIAS:
                # Stage bias through [1, 2048] chunks that share the x-tile
                # slots (a single [1, OUT] tile would hold 40 KB of column
                # space for the whole kernel), then replicate on GPSIMD.
                for m in range(N_REP):
                    W = 1808 if m == 4 else 2048
                    b1 = xp.tile([1, 2048], f32, tag="xt", name=f"b1_{m}")
                    nc.sync.dma_start(
                        b1[:, :W], bias_d[:, m * 2048 : m * 2048 + W]
                    )
                    nc.gpsimd.partition_broadcast(
                        biasr[:, m * 2048 : m * 2048 + W], b1[:, :W]
                    )
            else:
                nc.sync.dma_start(biasr[:], bias_d[:, :])

            for bt in range(nb):
                rows = slice(bt * P, (bt + 1) * P)

                x_t = xp.tile([P, IN], f32, tag="xt")
                nc.sync.dma_start(x_t[:], x_d[rows, :])

                # r0 = 1 / (sqrt(sum(x^2)) + eps), per batch row
                sq = sqp.tile([P, IN], f32, tag="sq")
                s = nrmp.tile([P, 1], f32, tag="s")
                nc.scalar.activation(sq[:], x_t[:], AF.Square, accum_out=s[:])
                t = nrmp.tile([P, 1], f32, tag="t")
                nc.scalar.activation(t[:], s[:], AF.Sqrt)
                t2 = nrmp.tile([P, 1], f32, tag="t2")
                nc.vector.tensor_scalar_add(t2[:], t[:], EPS)
                r0 = nrmp.tile([P, 1], f32, tag="r0")
                nc.vector.reciprocal(r0[:], t2[:])

                # WHT over the 4 input chunks (H4 stage), f32 -> bf16
                xc = [x_t[:, c * 512 : (c + 1) * 512] for c in range(4)]
                tt = [
                    wp.tile([P, 512], bf16, tag=f"t{k}", name=f"t{k}_{bt}")
                    for k in range(4)
                ]
                nc.vector.tensor_add(tt[0][:], xc[0], xc[1])
                nc.vector.tensor_sub(tt[1][:], xc[0], xc[1])
                nc.vector.tensor_add(tt[2][:], xc[2], xc[3])
                nc.vector.tensor_sub(tt[3][:], xc[2], xc[3])
                uu = [
                    wp.tile([P, 512], bf16, tag=f"u{k}", name=f"u{k}_{bt}")
                    for k in range(4)
                ]
                nc.vector.tensor_add(uu[0][:], tt[0][:], tt[2][:])
                nc.vector.tensor_sub(uu[2][:], tt[0][:], tt[2][:])
                nc.vector.tensor_add(uu[1][:], tt[1][:], tt[3][:])
                nc.vector.tensor_sub(uu[3][:], tt[1][:], tt[3][:])

                # Per output group jc: transpose 4 u-chunks (bf16, 1 cyc/row),
                # evacuate to SBUF, then 4-step PSUM accumulation matmul
                y0 = ypp.tile([P, 1024], f32, tag="y0", bufs=2)
                y1 = ypp.tile([P, 1024], f32, tag="y1", bufs=1)
                for jc in range(4):
                    pt = ptp.tile([P, 4, P], bf16, tag="pt")
                    for pc in range(4):
                        nc.tensor.matmul(
                            pt[:, pc, :],
                            uu[jc][:, pc * P : (pc + 1) * P],
                            identb[:],
                            is_transpose=True,
                            start=(pc == 0),
                            stop=(pc == 3),
                        )
                    uT = utp.tile([P, 4, P], bf16, tag="uT")
                    nc.scalar.copy(uT[:], pt[:])
                    ytile, off = (y0, jc * 512) if jc < 2 else (y1, (jc - 2) * 512)
                    for pc in range(4):
                        nc.tensor.matmul(
                            ytile[:, off : off + 512],
                            uT[:, pc, :],
                            lutb[:, pc, :],
                            start=(pc == 0),
                            stop=(pc == 3),
                        )

                # out_m = r0 * y + bias_m
                # DVE (PSUM src) for replicas 0-2; ACT evacuates ys = r0*y so
                # GPSIMD can add bias for replicas 3-4.  The last tile goes
                # all-DVE: GPSIMD's ~9us serial adds would otherwise sit on
                # the kernel tail.
                use_gp = GP_COMBINE and bt != nb - 1
                if use_gp:
                    ys = ysp.tile([P, IN], f32, tag="ys")
                    nc.scalar.activation(ys[:, 0:1024], y0[:], AF.Copy, scale=r0[:])
                    nc.scalar.activation(ys[:, 1024:2048], y1[:], AF.Copy, scale=r0[:])
                    dve_reps = (0, 1, 2)
                else:
                    dve_reps = (0, 1, 2, 3, 4)

                for m in dve_reps:
                    W = 1808 if m == 4 else 2048
                    st = stp.tile([P, 2048], f32, tag="st")
                    for half in range(2):
                        off = half * 1024
                        w = min(1024, W - off)
                        if w <= 0:
                            continue
                        yt = y0 if half == 0 else y1
                        nc.vector.scalar_tensor_tensor(
                            out=st[:, off : off + w],
                            in0=yt[:, :w],
                            scalar=r0[:],
                            in1=biasr[:, m * 2048 + off : m * 2048 + off + w],
                            op0=ALU.mult,
                            op1=ALU.add,
                        )
                    nc.sync.dma_start(out_d[rows, m * 2048 : m * 2048 + W], st[:, :W])

                if use_gp:
                    # Out-DMAs for the GPSIMD-produced replicas go out on the
                    # scalar HWDGE ring: the sync ring is strict FIFO, so a
                    # late GPSIMD add would otherwise head-of-line block the
                    # next tile's x load and DVE-produced outputs.
                    for m in (3, 4):
                        W = 1808 if m == 4 else 2048
                        st = stp.tile([P, 2048], f32, tag="st")
                        nc.gpsimd.tensor_add(
                            st[:, :W], ys[:, :W], biasr[:, m * 2048 : m * 2048 + W]
                        )
                        nc.scalar.dma_start(
                            out_d[rows, m * 2048 : m * 2048 + W], st[:, :W]
                        )

    nc.compile()
    return nc


def get_module(nb=NT):
    key = ("mod", nb, GP_COMBINE, BCAST_BIAS)
    if key not in _CACHE:
        _CACHE[key] = build_module(nb)
    return _CACHE[key]


def make_inputs(x, scale_val, bias):
    import ml_dtypes

    bf16 = ml_dtypes.bfloat16
    lutb = np.ascontiguousarray(make_lut(scale_val).astype(bf16))
    identb = np.ascontiguousarray(np.eye(P, dtype=np.float32).astype(bf16))
    bias = np.ascontiguousarray(bias.astype(np.float32))
    base = {"lutb": lutb, "identb": identb}
    if BCAST_BIAS:
        base["bias1"] = bias[None, :]
    else:
        base["biasr"] = np.ascontiguousarray(
            np.broadcast_to(bias[None, :], (P, OUT))
        )
    return [
        dict(base, x=x[c * B_CORE : (c + 1) * B_CORE]) for c in range(N_CORES)
    ]


def kernel(x, proj, scale, bias):
    from concourse.bass_utils import run_bass_kernel_spmd

    x = np.ascontiguousarray(np.asarray(x, dtype=np.float32))
    bias = np.asarray(bias, dtype=np.float32)
    scale_val = float(np.asarray(scale).reshape(-1)[0])
    del proj  # deterministic +-1 Hadamard; regenerated as the sign-pattern LUT

    nc = get_module()
    in_maps = make_inputs(x, scale_val, bias)
    res = run_bass_kernel_spmd(nc, in_maps, core_ids=list(range(N_CORES)))
    return np.concatenate([res.results[c]["out"] for c in range(N_CORES)], axis=0)


# revision 12
# speedup vs baseline: 1.1677x; 1.1677x over previous
"""Trainium2 Bass kernel for nn_HadamardProj.

Math:
    out = -scale * (x / (||x||_2 + 1e-8)) @ proj.T + bias
    proj[o, i] = (-1)^popcount(o & i)  for o < 10000, i < 2048.

Since i < 2^11, only the low 11 bits of o matter: proj[o, :] = H2048[o mod 2048, :]
where H2048 is the (symmetric) 2048-point Sylvester Hadamard matrix.  So the
[B,2048]x[2048,10000] projection collapses to a 2048-point transform
y = x @ H2048 plus column replication (10000 = 4*2048 + 1808):

    out[b, m*2048 + j] = r[b] * y[b, j] + bias[m*2048 + j],
    r = 1 / (||x_b|| + eps)        (-scale is baked into the LUT)

Kronecker split H2048 = H4 (x) H512 with i = c*512+p, j = jc*512+jp:

  1. ACT casts x to bf16 (xb).
  2. The H4 stage runs as signed transposes ON THE TENSOR ENGINE: regular
     bf16 matmuls with stationary = xb 128-chunk (c,pc) and moving = +-I,
     PSUM-accumulated over c:
        uT_(jc,pc)[p~, b] = sum_c H4[c,jc] * xb[b, c*512+pc*128+p~]
     (is_transpose mode ignores the rhs VALUES - probed - so signs need the
     regular-matmul path; 128-col streams at 1 cyc/row bf16.)  All 4 jc
     slices of one pc share a PSUM bank: the first matmul's start=True
     clears the bank, disjoint slices then overwrite, c>0 accumulates.
  3. 16 bf16 matmuls/tile: for group jc, accumulate over pc:
        y[b, jc*512+jp] += uT.T @ (-scale * H4[pc,q] H128)
     The rhs sign-pattern LUT has 4 entries (rows of H4), exactly +-scale in
     bf16.  bf16 stationaries get FWL weight loads that overlap in-flight
     matmuls (the fp32r path serialized 1280 LDWEIGHTS = 256us of PE time).
  4. Combine/replicate r*y+bias split across engines: replicas 0-2 as DVE
     scalar_tensor_tensor straight from PSUM; ACT evacuates ys = r*y to SBUF
     so GPSIMD tensor_add handles replicas 3-4 (tensor_tensor-class DVE ops
     never grab the shared DVE/GpSimd SBUF port pair, so no contention).
     This keeps DVE - the per-tile straggler otherwise - at ~8us/tile.

Precision: x and the WHT run through bf16 (three 2^-9-relative roundings);
the resulting max relative output error is ~2e-3 against a 2e-2 gate.

Sharding: data-parallel, 2048 batch rows per core across 8 cores. proj is
never read (regenerated as the 4-entry sign-pattern LUT host-side).
"""

import os
import sys

sys.path.insert(0, "/opt/trn_rl_repo")

import numpy as np

B_FULL = 16384
IN = 2048
OUT = 10000
N_CORES = 8
P = 128
B_CORE = B_FULL // N_CORES          # 2048 rows per core
NT = B_CORE // P                    # 16 row tiles per core
EPS = 1e-8
N_REP = 5                           # 4 full 2048 replicas + 1808 tail

# A/B flags (default = fastest known config)
GP_COMBINE = os.environ.get("HAD_GP", "1") == "1"      # replicas 3,4 on GPSIMD
BCAST_BIAS = os.environ.get("HAD_BCAST", "1") == "1"   # bias row + on-chip bcast

_CACHE = {}


def _popcount_parity(a):
    pc = np.zeros_like(a)
    n = int(a.max()).bit_length() if a.size else 1
    for k in range(max(n, 1)):
        pc += (a >> k) & 1
    return pc & 1


def _hadamard(n):
    i = np.arange(n, dtype=np.int64)
    return (1.0 - 2.0 * _popcount_parity(i[:, None] & i[None, :])).astype(np.float32)


def make_lut(scale_val):
    """4 sign-pattern rhs tensors [4, 128, 512]: row pc of H4 (x) H128, times -scale.

    lut[pc][:, q*128:(q+1)*128] = -scale * (-1)^popcount(pc & q) * H128
    """
    H128 = _hadamard(P)
    H4 = _hadamard(4)
    lut = np.empty((4, P, 512), dtype=np.float32)
    for pc in range(4):
        lut[pc] = np.concatenate([H4[pc, q] * H128 for q in range(4)], axis=1)
    return -scale_val * lut


def build_module(nb):
    """Build the per-core Bass module processing nb 128-row tiles."""
    import concourse.bass as bass  # noqa: F401
    from concourse import bacc
    import concourse.mybir as mybir
    import concourse.tile as tile

    f32 = mybir.dt.float32
    bf16 = mybir.dt.bfloat16
    AF = mybir.ActivationFunctionType
    ALU = mybir.AluOpType

    nc = bacc.Bacc("TRN2", target_bir_lowering=False, debug=False)
    H4 = _hadamard(4)
    x_d = nc.dram_tensor("x", [nb * P, IN], f32, kind="ExternalInput")
    lut_d = nc.dram_tensor("lutb", [4, P, 512], bf16, kind="ExternalInput")
    ident_d = nc.dram_tensor("identb", [2, P, P], bf16, kind="ExternalInput")
    if BCAST_BIAS:
        bias_d = nc.dram_tensor("bias1", [1, OUT], f32, kind="ExternalInput")
    else:
        bias_d = nc.dram_tensor("biasr", [P, OUT], f32, kind="ExternalInput")
    out_d = nc.dram_tensor("out", [nb * P, OUT], f32, kind="ExternalOutput")

    with tile.TileContext(nc) as tc:
        with (
            tc.tile_pool(name="const", bufs=1) as cp,
            tc.tile_pool(name="xin", bufs=6) as xp,
            tc.tile_pool(name="xbp", bufs=2) as xbp,
            tc.tile_pool(name="sq", bufs=2) as sqp,
            tc.tile_pool(name="ut", bufs=2) as utp,
            tc.tile_pool(name="ysb", bufs=2) as ysp,
            tc.tile_pool(name="nrm", bufs=8) as nrmp,
            tc.tile_pool(name="stage", bufs=7) as stp,
            tc.tile_pool(name="pt", bufs=2, space="PSUM") as ptp,
            tc.tile_pool(name="yp", bufs=3, space="PSUM") as ypp,
        ):
            identb = cp.tile([P, 2, P], bf16, tag="identb")
            nc.sync.dma_start(identb[:], ident_d[:, :, :].rearrange("s p n -> p s n"))
            lutb = cp.tile([P, 4, 512], bf16, tag="lutb")
            nc.sync.dma_start(lutb[:], lut_d[:, :, :].rearrange("g p n -> p g n"))
            biasr = cp.tile([P, OUT], f32, tag="biasr")
            if BCAST_BIAS:
                # Stage bias through [1, 2048] chunks that share the x-tile
                # slots (a single [1, OUT] tile would hold 40 KB of column
                # space for the whole kernel), then replicate on GPSIMD.
                for m in range(N_REP):
                    W = 1808 if m == 4 else 2048
                    b1 = xp.tile([1, 2048], f32, tag="xt", name=f"b1_{m}")
                    nc.sync.dma_start(
                        b1[:, :W], bias_d[:, m * 2048 : m * 2048 + W]
                    )
                    nc.gpsimd.partition_broadcast(
                        biasr[:, m * 2048 : m * 2048 + W], b1[:, :W]
                    )
            else:
                nc.sync.dma_start(biasr[:], bias_d[:, :])

            for bt in range(nb):
                rows = slice(bt * P, (bt + 1) * P)

                x_t = xp.tile([P, IN], f32, tag="xt")
                nc.sync.dma_start(x_t[:], x_d[rows, :])

                # r0 = 1 / (sqrt(sum(x^2)) + eps), per batch row
                sq = sqp.tile([P, IN], f32, tag="sq")
                s = nrmp.tile([P, 1], f32, tag="s")
                nc.scalar.activation(sq[:], x_t[:], AF.Square, accum_out=s[:])
                t = nrmp.tile([P, 1], f32, tag="t")
                nc.scalar.activation(t[:], s[:], AF.Sqrt)
                t2 = nrmp.tile([P, 1], f32, tag="t2")
                nc.vector.tensor_scalar_add(t2[:], t[:], EPS)
                r0 = nrmp.tile([P, 1], f32, tag="r0")
                nc.vector.reciprocal(r0[:], t2[:])

                # bf16 cast for the PE stage
                xb = xbp.tile([P, IN], bf16, tag="xb")
                nc.scalar.copy(xb[:], x_t[:])

                # H4 stage as signed transposes on PE: for each pc, one PSUM
                # bank holds the 4 jc-slices; each slice accumulates
                # sum_c H4[c,jc] * xb_chunk(c,pc).T via moving-operand +-I.
                # Weights (xb chunk) stay loaded across the 4 jc matmuls.
                uTs = []
                for pc in range(4):
                    pt = ptp.tile([P, 4, P], f32, tag="pt", name=f"pt{pc}_{bt}")
                    for c in range(4):
                        lhs = xb[:, c * 512 + pc * P : c * 512 + (pc + 1) * P]
                        for jc in range(4):
                            sgn = 1 if H4[c, jc] > 0 else 0
                            nc.tensor.matmul(
                                pt[:, jc, :],
                                lhs,
                                identb[:, 1 - sgn, :],
                                start=(c == 0 and jc == 0),
                                stop=(c == 3 and jc == 3),
                            )
                    uT = utp.tile(
                        [P, 4, P], bf16, tag=f"uT{pc}", name=f"uT{pc}_{bt}"
                    )
                    nc.scalar.copy(uT[:], pt[:])
                    uTs.append(uT)

                # H512 stage: per group jc, accumulate over pc
                y0 = ypp.tile([P, 1024], f32, tag="y0", bufs=2)
                y1 = ypp.tile([P, 1024], f32, tag="y1", bufs=1)
                for jc in range(4):
                    ytile, off = (y0, jc * 512) if jc < 2 else (y1, (jc - 2) * 512)
                    for pc in range(4):
                        nc.tensor.matmul(
                            ytile[:, off : off + 512],
                            uTs[pc][:, jc, :],
                            lutb[:, pc, :],
                            start=(pc == 0),
                            stop=(pc == 3),
                        )

                # out_m = r0 * y + bias_m
                # DVE (PSUM src) for replicas 0-2; ACT evacuates ys = r0*y so
                # GPSIMD can add bias for replicas 3-4.  The last tile goes
                # all-DVE: GPSIMD's ~9us serial adds would otherwise sit on
                # the kernel tail.
                use_gp = GP_COMBINE and bt != nb - 1
                if use_gp:
                    ys = ysp.tile([P, IN], f32, tag="ys")
                    nc.scalar.activation(ys[:, 0:1024], y0[:], AF.Copy, scale=r0[:])
                    nc.scalar.activation(ys[:, 1024:2048], y1[:], AF.Copy, scale=r0[:])
                    dve_reps = (0, 1, 2)
                else:
                    dve_reps = (0, 1, 2, 3, 4)

                for m in dve_reps:
                    W = 1808 if m == 4 else 2048
                    st = stp.tile([P, 2048], f32, tag="st")
                    for half in range(2):
                        off = half * 1024
                        w = min(1024, W - off)
                        if w <= 0:
                            continue
                        yt = y0 if half == 0 else y1
                        nc.vector.scalar_tensor_tensor(
                            out=st[:, off : off + w],
                            in0=yt[:, :w],
                            scalar=r0[:],
                            in1=biasr[:, m * 2048 + off : m * 2048 + off + w],
                            op0=ALU.mult,
                            op1=ALU.add,
                        )
                    nc.sync.dma_start(out_d[rows, m * 2048 : m * 2048 + W], st[:, :W])

                if use_gp:
                    # Out-DMAs for the GPSIMD-produced replicas go out on the
                    # scalar HWDGE ring: the sync ring is strict FIFO, so a
                    # late GPSIMD add would otherwise head-of-line block the
                    # next tile's x load and DVE-produced outputs.
                    for m in (3, 4):
                        W = 1808 if m == 4 else 2048
                        st = stp.tile([P, 2048], f32, tag="st")
                        nc.gpsimd.tensor_add(
                            st[:, :W], ys[:, :W], biasr[:, m * 2048 : m * 2048 + W]
                        )
                        nc.scalar.dma_start(
                            out_d[rows, m * 2048 : m * 2048 + W], st[:, :W]
                        )

    nc.compile()
    return nc


def get_module(nb=NT):
    key = ("mod", nb, GP_COMBINE, BCAST_BIAS)
    if key not in _CACHE:
        _CACHE[key] = build_module(nb)
    return _CACHE[key]


def make_inputs(x, scale_val, bias):
    import ml_dtypes

    bf16 = ml_dtypes.bfloat16
    lutb = np.ascontiguousarray(make_lut(scale_val).astype(bf16))
    eye = np.eye(P, dtype=np.float32)
    identb = np.ascontiguousarray(np.stack([eye, -eye]).astype(bf16))
    bias = np.ascontiguousarray(bias.astype(np.float32))
    base = {"lutb": lutb, "identb": identb}
    if BCAST_BIAS:
        base["bias1"] = bias[None, :]
    else:
        base["biasr"] = np.ascontiguousarray(
            np.broadcast_to(bias[None, :], (P, OUT))
        )
    return [
        dict(base, x=x[c * B_CORE : (c + 1) * B_CORE]) for c in range(N_CORES)
    ]


def kernel(x, proj, scale, bias):
    from concourse.bass_utils import run_bass_kernel_spmd

    x = np.ascontiguousarray(np.asarray(x, dtype=np.float32))
    bias = np.asarray(bias, dtype=np.float32)
    scale_val = float(np.asarray(scale).reshape(-1)[0])
    del proj  # deterministic +-1 Hadamard; regenerated as the sign-pattern LUT

    nc = get_module()
    in_maps = make_inputs(x, scale_val, bias)
    res = run_bass_kernel_spmd(nc, in_maps, core_ids=list(range(N_CORES)))
    return np.concatenate([res.results[c]["out"] for c in range(N_CORES)], axis=0)


# revision 21
# speedup vs baseline: 1.2010x; 1.0285x over previous
"""Trainium2 Bass kernel for nn_HadamardProj.

Math:
    out = -scale * (x / (||x||_2 + 1e-8)) @ proj.T + bias
    proj[o, i] = (-1)^popcount(o & i)  for o < 10000, i < 2048.

Since i < 2^11, only the low 11 bits of o matter: proj[o, :] = H2048[o mod 2048, :]
where H2048 is the (symmetric) 2048-point Sylvester Hadamard matrix.  So the
[B,2048]x[2048,10000] projection collapses to a 2048-point transform
y = x @ H2048 plus column replication (10000 = 4*2048 + 1808):

    out[b, m*2048 + j] = r[b] * y[b, j] + bias[m*2048 + j],
    r = 1 / (||x_b|| + eps)        (-scale is baked into the LUT)

Kronecker split H2048 = H4 (x) H512 with i = c*512+p, j = jc*512+jp:

  1. ACT casts x to bf16 (xb).
  2. The H4 stage runs as signed transposes ON THE TENSOR ENGINE: regular
     bf16 matmuls with stationary = xb 128-chunk (c,pc) and moving = +-I,
     PSUM-accumulated over c:
        uT_(jc,pc)[p~, b] = sum_c H4[c,jc] * xb[b, c*512+pc*128+p~]
     (is_transpose mode ignores the rhs VALUES - probed - so signs need the
     regular-matmul path; 128-col streams at 1 cyc/row bf16.)  All 4 jc
     slices of one pc share a PSUM bank: the first matmul's start=True
     clears the bank, disjoint slices then overwrite, c>0 accumulates.
  3. 16 bf16 matmuls/tile: for group jc, accumulate over pc:
        y[b, jc*512+jp] += uT.T @ (-scale * H4[pc,q] H128)
     The rhs sign-pattern LUT has 4 entries (rows of H4), exactly +-scale in
     bf16.  bf16 stationaries get FWL weight loads that overlap in-flight
     matmuls (the fp32r path serialized 1280 LDWEIGHTS = 256us of PE time).
  4. Combine/replicate r*y+bias split across engines: replicas 0-2 as DVE
     scalar_tensor_tensor straight from PSUM; ACT evacuates ys = r*y to SBUF
     so GPSIMD tensor_add handles replicas 3-4 (tensor_tensor-class DVE ops
     never grab the shared DVE/GpSimd SBUF port pair, so no contention).
     This keeps DVE - the per-tile straggler otherwise - at ~8us/tile.

Precision: x and the WHT run through bf16 (three 2^-9-relative roundings);
the resulting max relative output error is ~2e-3 against a 2e-2 gate.

Sharding: data-parallel, 2048 batch rows per core across 8 cores. proj is
never read (regenerated as the 4-entry sign-pattern LUT host-side).
"""

import os
import sys

sys.path.insert(0, "/opt/trn_rl_repo")

import numpy as np

B_FULL = 16384
IN = 2048
OUT = 10000
N_CORES = 8
P = 128
B_CORE = B_FULL // N_CORES          # 2048 rows per core
NT = B_CORE // P                    # 16 row tiles per core
EPS = 1e-8
N_REP = 5                           # 4 full 2048 replicas + 1808 tail

# A/B flags (default = fastest known config)
GP_COMBINE = os.environ.get("HAD_GP", "1") == "1"      # replicas 3,4 on GPSIMD
BCAST_BIAS = os.environ.get("HAD_BCAST", "1") == "1"   # bias row + on-chip bcast

_CACHE = {}


def _popcount_parity(a):
    pc = np.zeros_like(a)
    n = int(a.max()).bit_length() if a.size else 1
    for k in range(max(n, 1)):
        pc += (a >> k) & 1
    return pc & 1


def _hadamard(n):
    i = np.arange(n, dtype=np.int64)
    return (1.0 - 2.0 * _popcount_parity(i[:, None] & i[None, :])).astype(np.float32)


def make_lut(scale_val):
    """4 sign-pattern rhs tensors [4, 128, 512]: row pc of H4 (x) H128, times -scale.

    lut[pc][:, q*128:(q+1)*128] = -scale * (-1)^popcount(pc & q) * H128
    """
    H128 = _hadamard(P)
    H4 = _hadamard(4)
    lut = np.empty((4, P, 512), dtype=np.float32)
    for pc in range(4):
        lut[pc] = np.concatenate([H4[pc, q] * H128 for q in range(4)], axis=1)
    return -scale_val * lut


def build_module(nb):
    """Build the per-core Bass module processing nb 128-row tiles."""
    import concourse.bass as bass  # noqa: F401
    from concourse import bacc
    import concourse.mybir as mybir
    import concourse.tile as tile

    f32 = mybir.dt.float32
    bf16 = mybir.dt.bfloat16
    AF = mybir.ActivationFunctionType
    ALU = mybir.AluOpType

    nc = bacc.Bacc("TRN2", target_bir_lowering=False, debug=False)
    H4 = _hadamard(4)
    assert nb % 4 == 0
    nsb = nb // 4
    # x arrives host-cast to bf16 (the kernel rounds x to bf16 anyway before
    # the PE stage, so this costs no extra precision) and is loaded in 4-tile
    # 2MB superblocks: 1MB dma_starts only reach ~340GB/s of the ~358 HBM
    # peak; 2MB gets ~395.
    x_d = nc.dram_tensor("x", [nsb, 4, P, IN], bf16, kind="ExternalInput")
    lut_d = nc.dram_tensor("lutb", [4, P, 512], bf16, kind="ExternalInput")
    ident_d = nc.dram_tensor("identb", [2, P, P], bf16, kind="ExternalInput")
    if BCAST_BIAS:
        bias_d = nc.dram_tensor("bias1", [1, OUT], f32, kind="ExternalInput")
    else:
        bias_d = nc.dram_tensor("biasr", [P, OUT], f32, kind="ExternalInput")
    out_d = nc.dram_tensor("out", [nb * P, OUT], f32, kind="ExternalOutput")

    with tile.TileContext(nc) as tc:
        with (
            tc.tile_pool(name="const", bufs=1) as cp,
            tc.tile_pool(name="xin", bufs=3) as xp,
            tc.tile_pool(name="b1p", bufs=2) as b1p,
            tc.tile_pool(name="sq", bufs=2) as sqp,
            tc.tile_pool(name="ut", bufs=2) as utp,
            tc.tile_pool(name="ysb", bufs=2) as ysp,
            tc.tile_pool(name="nrm", bufs=8) as nrmp,
            tc.tile_pool(name="stage", bufs=6) as stp,
            tc.tile_pool(name="pt", bufs=2, space="PSUM") as ptp,
            tc.tile_pool(name="yp", bufs=3, space="PSUM") as ypp,
        ):
            identb = cp.tile([P, 2, P], bf16, tag="identb")
            nc.sync.dma_start(identb[:], ident_d[:, :, :].rearrange("s p n -> p s n"))
            lutb = cp.tile([P, 4, 512], bf16, tag="lutb")
            nc.sync.dma_start(lutb[:], lut_d[:, :, :].rearrange("g p n -> p g n"))
            biasr = cp.tile([P, OUT], f32, tag="biasr")
            if BCAST_BIAS:
                # Stage bias through [1, 2048] chunks (a single [1, OUT] tile
                # would hold 40 KB of column space for the whole kernel),
                # then replicate across partitions on GPSIMD.
                for m in range(N_REP):
                    W = 1808 if m == 4 else 2048
                    b1 = b1p.tile([1, 2048], f32, tag="b1", name=f"b1_{m}")
                    nc.sync.dma_start(
                        b1[:, :W], bias_d[:, m * 2048 : m * 2048 + W]
                    )
                    nc.gpsimd.partition_broadcast(
                        biasr[:, m * 2048 : m * 2048 + W], b1[:, :W]
                    )
            else:
                nc.sync.dma_start(biasr[:], bias_d[:, :])

            xsbs = [None] * nsb
            for bt in range(nb):
                rows = slice(bt * P, (bt + 1) * P)
                sb, ti = divmod(bt, 4)
                if ti == 0:
                    xsb = xp.tile([P, 4, IN], bf16, tag="xt", name=f"xsb{sb}")
                    nc.sync.dma_start(
                        xsb[:], x_d[sb, :, :, :].rearrange("t p n -> p t n")
                    )
                    xsbs[sb] = xsb
                xt = xsbs[sb]
                x_t = xt[:, ti, :]

                # r0 = 1 / (sqrt(sum(x^2)) + eps), per batch row
                sq = sqp.tile([P, IN], f32, tag="sq")
                s = nrmp.tile([P, 1], f32, tag="s")
                nc.scalar.activation(sq[:], x_t, AF.Square, accum_out=s[:])
                t = nrmp.tile([P, 1], f32, tag="t")
                nc.scalar.activation(t[:], s[:], AF.Sqrt)
                t2 = nrmp.tile([P, 1], f32, tag="t2")
                nc.vector.tensor_scalar_add(t2[:], t[:], EPS)
                r0 = nrmp.tile([P, 1], f32, tag="r0")
                nc.vector.reciprocal(r0[:], t2[:])

                # H4 stage as signed transposes on PE: for each pc, one PSUM
                # bank holds the 4 jc-slices; each slice accumulates
                # sum_c H4[c,jc] * xb_chunk(c,pc).T via moving-operand +-I.
                # Weights (xb chunk) stay loaded across the 4 jc matmuls.
                uTs = []
                for pc in range(4):
                    pt = ptp.tile([P, 4, P], f32, tag="pt", name=f"pt{pc}_{bt}")
                    for c in range(4):
                        lhs = xt[:, ti, c * 512 + pc * P : c * 512 + (pc + 1) * P]
                        for jc in range(4):
                            sgn = 1 if H4[c, jc] > 0 else 0
                            nc.tensor.matmul(
                                pt[:, jc, :],
                                lhs,
                                identb[:, 1 - sgn, :],
                                start=(c == 0 and jc == 0),
                                stop=(c == 3 and jc == 3),
                            )
                    uT = utp.tile(
                        [P, 4, P], bf16, tag=f"uT{pc}", name=f"uT{pc}_{bt}"
                    )
                    nc.scalar.copy(uT[:], pt[:])
                    uTs.append(uT)

                # H512 stage: per group jc, accumulate over pc
                y0 = ypp.tile([P, 1024], f32, tag="y0", bufs=2)
                y1 = ypp.tile([P, 1024], f32, tag="y1", bufs=1)
                for jc in range(4):
                    ytile, off = (y0, jc * 512) if jc < 2 else (y1, (jc - 2) * 512)
                    for pc in range(4):
                        nc.tensor.matmul(
                            ytile[:, off : off + 512],
                            uTs[pc][:, jc, :],
                            lutb[:, pc, :],
                            start=(pc == 0),
                            stop=(pc == 3),
                        )

                # out_m = r0 * y + bias_m
                # DVE (PSUM src) for replicas 0-2; ACT evacuates ys = r0*y so
                # GPSIMD can add bias for replicas 3-4.  The last tile goes
                # all-DVE: GPSIMD's ~9us serial adds would otherwise sit on
                # the kernel tail.
                use_gp = GP_COMBINE and bt != nb - 1
                if use_gp:
                    ys = ysp.tile([P, IN], f32, tag="ys")
                    nc.scalar.activation(ys[:, 0:1024], y0[:], AF.Copy, scale=r0[:])
                    nc.scalar.activation(ys[:, 1024:2048], y1[:], AF.Copy, scale=r0[:])
                    dve_reps = (0, 1, 2)
                else:
                    dve_reps = (0, 1, 2, 3, 4)

                for m in dve_reps:
                    W = 1808 if m == 4 else 2048
                    st = stp.tile([P, 2048], f32, tag="st")
                    for half in range(2):
                        off = half * 1024
                        w = min(1024, W - off)
                        if w <= 0:
                            continue
                        yt = y0 if half == 0 else y1
                        nc.vector.scalar_tensor_tensor(
                            out=st[:, off : off + w],
                            in0=yt[:, :w],
                            scalar=r0[:],
                            in1=biasr[:, m * 2048 + off : m * 2048 + off + w],
                            op0=ALU.mult,
                            op1=ALU.add,
                        )
                    nc.sync.dma_start(out_d[rows, m * 2048 : m * 2048 + W], st[:, :W])

                if use_gp:
                    # Out-DMAs for the GPSIMD-produced replicas go out on the
                    # scalar HWDGE ring: the sync ring is strict FIFO, so a
                    # late GPSIMD add would otherwise head-of-line block the
                    # next tile's x load and DVE-produced outputs.
                    for m in (3, 4):
                        W = 1808 if m == 4 else 2048
                        st = stp.tile([P, 2048], f32, tag="st")
                        nc.gpsimd.tensor_add(
                            st[:, :W], ys[:, :W], biasr[:, m * 2048 : m * 2048 + W]
                        )
                        nc.scalar.dma_start(
                            out_d[rows, m * 2048 : m * 2048 + W], st[:, :W]
                        )

    nc.compile()
    return nc


def get_module(nb=NT):
    key = ("mod", nb, GP_COMBINE, BCAST_BIAS)
    if key not in _CACHE:
        _CACHE[key] = build_module(nb)
    return _CACHE[key]


def make_inputs(x, scale_val, bias):
    import ml_dtypes

    bf16 = ml_dtypes.bfloat16
    lutb = np.ascontiguousarray(make_lut(scale_val).astype(bf16))
    eye = np.eye(P, dtype=np.float32)
    identb = np.ascontiguousarray(np.stack([eye, -eye]).astype(bf16))
    bias = np.ascontiguousarray(bias.astype(np.float32))
    base = {"lutb": lutb, "identb": identb}
    if BCAST_BIAS:
        base["bias1"] = bias[None, :]
    else:
        base["biasr"] = np.ascontiguousarray(
            np.broadcast_to(bias[None, :], (P, OUT))
        )
    xb = np.ascontiguousarray(x.astype(bf16)).reshape(N_CORES, NT // 4, 4, P, IN)
    return [dict(base, x=xb[c]) for c in range(N_CORES)]


def kernel(x, proj, scale, bias):
    from concourse.bass_utils import run_bass_kernel_spmd

    x = np.ascontiguousarray(np.asarray(x, dtype=np.float32))
    bias = np.asarray(bias, dtype=np.float32)
    scale_val = float(np.asarray(scale).reshape(-1)[0])
    del proj  # deterministic +-1 Hadamard; regenerated as the sign-pattern LUT

    nc = get_module()
    in_maps = make_inputs(x, scale_val, bias)
    res = run_bass_kernel_spmd(nc, in_maps, core_ids=list(range(N_CORES)))
    return np.concatenate([res.results[c]["out"] for c in range(N_CORES)], axis=0)


# revision 27
# speedup vs baseline: 1.2442x; 1.0360x over previous
"""Trainium2 Bass kernel for nn_HadamardProj.

Math:
    out = -scale * (x / (||x||_2 + 1e-8)) @ proj.T + bias
    proj[o, i] = (-1)^popcount(o & i)  for o < 10000, i < 2048.

Since i < 2^11, only the low 11 bits of o matter: proj[o, :] = H2048[o mod 2048, :]
where H2048 is the (symmetric) 2048-point Sylvester Hadamard matrix.  So the
[B,2048]x[2048,10000] projection collapses to a 2048-point transform
y = x @ H2048 plus column replication (10000 = 4*2048 + 1808):

    out[b, m*2048 + j] = r[b] * y[b, j] + bias[m*2048 + j],
    r = 1 / (||x_b|| + eps)        (-scale is baked into the LUT)

Kronecker split H2048 = H4 (x) H512 with i = c*512+p, j = jc*512+jp:

  1. x arrives host-cast to bf16 (numerically free: the kernel would round x
     to bf16 before the PE stage anyway) and is DMA'd in 2MB 4-tile
     superblocks - halves the x HBM traffic and improves DMA efficiency.
  2. The H4 stage runs as signed transposes ON THE TENSOR ENGINE: regular
     bf16 matmuls with stationary = x 128-chunk (c,pc) and moving = +-I,
     PSUM-accumulated over c:
        uT_(jc,pc)[p~, b] = sum_c H4[c,jc] * x[b, c*512+pc*128+p~]
     (is_transpose mode ignores the rhs VALUES - probed - so signs need the
     regular-matmul path; 128-col streams at 1 cyc/row bf16.)  All 4 jc
     slices of one pc share a PSUM bank: the first matmul's start=True
     clears the bank, disjoint slices then overwrite, c>0 accumulates.
  3. 16 bf16 matmuls/tile: for group jc, accumulate over pc:
        y[b, jc*512+jp] += uT.T @ (-scale * H4[pc,q] H128)
     The rhs sign-pattern LUT has 4 entries (rows of H4), exactly +-scale in
     bf16.  bf16 stationaries get FWL weight loads that overlap in-flight
     matmuls (the fp32r path serialized 1280 LDWEIGHTS = 256us of PE time).
  4. Combine/replicate r*y+bias split across engines: replicas 0-2 as DVE
     scalar_tensor_tensor straight from PSUM; ACT evacuates ys = r*y to SBUF
     so GPSIMD tensor_add handles replicas 3-4 (tensor_tensor-class DVE ops
     never grab the shared DVE/GpSimd SBUF port pair, so no contention).
     This keeps DVE - the per-tile straggler otherwise - at ~8us/tile.

Precision: x and the WHT run through bf16 (three 2^-9-relative roundings);
the resulting max relative output error is ~2e-3 against a 2e-2 gate.

Sharding: data-parallel, 2048 batch rows per core across 8 cores. proj is
never read (regenerated as the 4-entry sign-pattern LUT host-side).
"""

import os
import sys

sys.path.insert(0, "/opt/trn_rl_repo")

import numpy as np

B_FULL = 16384
IN = 2048
OUT = 10000
N_CORES = 8
P = 128
B_CORE = B_FULL // N_CORES          # 2048 rows per core
NT = B_CORE // P                    # 16 row tiles per core
EPS = 1e-8
N_REP = 5                           # 4 full 2048 replicas + 1808 tail

# A/B flags (default = fastest known config)
GP_COMBINE = os.environ.get("HAD_GP", "1") == "1"      # replicas 3,4 on GPSIMD
BCAST_BIAS = os.environ.get("HAD_BCAST", "1") == "1"   # bias row + on-chip bcast

_CACHE = {}


def _popcount_parity(a):
    pc = np.zeros_like(a)
    n = int(a.max()).bit_length() if a.size else 1
    for k in range(max(n, 1)):
        pc += (a >> k) & 1
    return pc & 1


def _hadamard(n):
    i = np.arange(n, dtype=np.int64)
    return (1.0 - 2.0 * _popcount_parity(i[:, None] & i[None, :])).astype(np.float32)


def make_lut(scale_val):
    """4 sign-pattern rhs tensors [4, 128, 512]: row pc of H4 (x) H128, times -scale.

    lut[pc][:, q*128:(q+1)*128] = -scale * (-1)^popcount(pc & q) * H128
    """
    H128 = _hadamard(P)
    H4 = _hadamard(4)
    lut = np.empty((4, P, 512), dtype=np.float32)
    for pc in range(4):
        lut[pc] = np.concatenate([H4[pc, q] * H128 for q in range(4)], axis=1)
    return -scale_val * lut


def build_module(nb):
    """Build the per-core Bass module processing nb 128-row tiles."""
    import concourse.bass as bass  # noqa: F401
    from concourse import bacc
    import concourse.mybir as mybir
    import concourse.tile as tile

    f32 = mybir.dt.float32
    bf16 = mybir.dt.bfloat16
    AF = mybir.ActivationFunctionType
    ALU = mybir.AluOpType

    nc = bacc.Bacc("TRN2", target_bir_lowering=False, debug=False)
    H4 = _hadamard(4)
    assert nb % 4 == 0
    nsb = nb // 4
    # x arrives host-cast to bf16 (the kernel rounds x to bf16 anyway before
    # the PE stage, so this costs no extra precision) and is loaded in 4-tile
    # 2MB superblocks: 1MB dma_starts only reach ~340GB/s of the ~358 HBM
    # peak; 2MB gets ~395.
    x_d = nc.dram_tensor("x", [nsb, 4, P, IN], bf16, kind="ExternalInput")
    lut_d = nc.dram_tensor("lutb", [4, P, 512], bf16, kind="ExternalInput")
    ident_d = nc.dram_tensor("identb", [2, P, P], bf16, kind="ExternalInput")
    if BCAST_BIAS:
        bias_d = nc.dram_tensor("bias1", [1, OUT], f32, kind="ExternalInput")
    else:
        bias_d = nc.dram_tensor("biasr", [P, OUT], f32, kind="ExternalInput")
    out_d = nc.dram_tensor("out", [nb * P, OUT], f32, kind="ExternalOutput")

    with tile.TileContext(nc) as tc:
        with (
            tc.tile_pool(name="const", bufs=1) as cp,
            tc.tile_pool(name="xin", bufs=3) as xp,
            tc.tile_pool(name="b1p", bufs=2) as b1p,
            tc.tile_pool(name="sq", bufs=2) as sqp,
            tc.tile_pool(name="ut", bufs=2) as utp,
            tc.tile_pool(name="ysb", bufs=2) as ysp,
            tc.tile_pool(name="nrm", bufs=8) as nrmp,
            tc.tile_pool(name="stage", bufs=6) as stp,
            tc.tile_pool(name="pt", bufs=2, space="PSUM") as ptp,
            tc.tile_pool(name="yp", bufs=3, space="PSUM") as ypp,
        ):
            identb = cp.tile([P, 2, P], bf16, tag="identb")
            nc.sync.dma_start(identb[:], ident_d[:, :, :].rearrange("s p n -> p s n"))
            lutb = cp.tile([P, 4, 512], bf16, tag="lutb")
            nc.sync.dma_start(lutb[:], lut_d[:, :, :].rearrange("g p n -> p g n"))
            biasr = cp.tile([P, OUT], f32, tag="biasr")
            if BCAST_BIAS:
                # Stage bias through [1, 2048] chunks (a single [1, OUT] tile
                # would hold 40 KB of column space for the whole kernel),
                # then replicate across partitions on GPSIMD.
                for m in range(N_REP):
                    W = 1808 if m == 4 else 2048
                    b1 = b1p.tile([1, 2048], f32, tag="b1", name=f"b1_{m}")
                    nc.sync.dma_start(
                        b1[:, :W], bias_d[:, m * 2048 : m * 2048 + W]
                    )
                    nc.gpsimd.partition_broadcast(
                        biasr[:, m * 2048 : m * 2048 + W], b1[:, :W]
                    )
            else:
                nc.sync.dma_start(biasr[:], bias_d[:, :])

            xsbs = [None] * nsb
            for bt in range(nb):
                rows = slice(bt * P, (bt + 1) * P)
                sb, ti = divmod(bt, 4)
                if ti == 0:
                    xsb = xp.tile([P, 4, IN], bf16, tag="xt", name=f"xsb{sb}")
                    nc.sync.dma_start(
                        xsb[:], x_d[sb, :, :, :].rearrange("t p n -> p t n")
                    )
                    xsbs[sb] = xsb
                xt = xsbs[sb]
                x_t = xt[:, ti, :]

                # r0 = 1 / (sqrt(sum(x^2)) + eps), per batch row
                sq = sqp.tile([P, IN], f32, tag="sq")
                s = nrmp.tile([P, 1], f32, tag="s")
                nc.scalar.activation(sq[:], x_t, AF.Square, accum_out=s[:])
                t = nrmp.tile([P, 1], f32, tag="t")
                nc.scalar.activation(t[:], s[:], AF.Sqrt)
                t2 = nrmp.tile([P, 1], f32, tag="t2")
                nc.vector.tensor_scalar_add(t2[:], t[:], EPS)
                r0 = nrmp.tile([P, 1], f32, tag="r0")
                nc.vector.reciprocal(r0[:], t2[:])

                # H4 stage as signed transposes on PE: for each pc, one PSUM
                # bank holds the 4 jc-slices; each slice accumulates
                # sum_c H4[c,jc] * xb_chunk(c,pc).T via moving-operand +-I.
                # Weights (xb chunk) stay loaded across the 4 jc matmuls.
                uTs = []
                for pc in range(4):
                    pt = ptp.tile([P, 4, P], f32, tag="pt", name=f"pt{pc}_{bt}")
                    for c in range(4):
                        lhs = xt[:, ti, c * 512 + pc * P : c * 512 + (pc + 1) * P]
                        for jc in range(4):
                            sgn = 1 if H4[c, jc] > 0 else 0
                            nc.tensor.matmul(
                                pt[:, jc, :],
                                lhs,
                                identb[:, 1 - sgn, :],
                                start=(c == 0 and jc == 0),
                                stop=(c == 3 and jc == 3),
                            )
                    uT = utp.tile(
                        [P, 4, P], bf16, tag=f"uT{pc}", name=f"uT{pc}_{bt}"
                    )
                    nc.scalar.copy(uT[:], pt[:])
                    uTs.append(uT)

                # H512 stage: per group jc, accumulate over pc
                y0 = ypp.tile([P, 1024], f32, tag="y0", bufs=2)
                y1 = ypp.tile([P, 1024], f32, tag="y1", bufs=1)
                for jc in range(4):
                    ytile, off = (y0, jc * 512) if jc < 2 else (y1, (jc - 2) * 512)
                    for pc in range(4):
                        nc.tensor.matmul(
                            ytile[:, off : off + 512],
                            uTs[pc][:, jc, :],
                            lutb[:, pc, :],
                            start=(pc == 0),
                            stop=(pc == 3),
                        )

                # out_m = r0 * y + bias_m
                # DVE (PSUM src) for replicas 0-2; ACT evacuates ys = r0*y so
                # GPSIMD can add bias for replicas 3-4.  The last tile goes
                # all-DVE: GPSIMD's ~9us serial adds would otherwise sit on
                # the kernel tail.
                use_gp = GP_COMBINE and bt != nb - 1
                if use_gp:
                    ys = ysp.tile([P, IN], f32, tag="ys")
                    nc.scalar.activation(ys[:, 0:1024], y0[:], AF.Copy, scale=r0[:])
                    nc.scalar.activation(ys[:, 1024:2048], y1[:], AF.Copy, scale=r0[:])
                    dve_reps = (0, 1, 2)
                else:
                    dve_reps = (0, 1, 2, 3, 4)

                for m in dve_reps:
                    W = 1808 if m == 4 else 2048
                    st = stp.tile([P, 2048], f32, tag="st")
                    for half in range(2):
                        off = half * 1024
                        w = min(1024, W - off)
                        if w <= 0:
                            continue
                        yt = y0 if half == 0 else y1
                        nc.vector.scalar_tensor_tensor(
                            out=st[:, off : off + w],
                            in0=yt[:, :w],
                            scalar=r0[:],
                            in1=biasr[:, m * 2048 + off : m * 2048 + off + w],
                            op0=ALU.mult,
                            op1=ALU.add,
                        )
                    nc.sync.dma_start(out_d[rows, m * 2048 : m * 2048 + W], st[:, :W])

                if use_gp:
                    # Out-DMAs for the GPSIMD-produced replicas go out on the
                    # scalar HWDGE ring: the sync ring is strict FIFO, so a
                    # late GPSIMD add would otherwise head-of-line block the
                    # next tile's x load and DVE-produced outputs.
                    for m in (3, 4):
                        W = 1808 if m == 4 else 2048
                        st = stp.tile([P, 2048], f32, tag="st")
                        nc.gpsimd.tensor_add(
                            st[:, :W], ys[:, :W], biasr[:, m * 2048 : m * 2048 + W]
                        )
                        nc.scalar.dma_start(
                            out_d[rows, m * 2048 : m * 2048 + W], st[:, :W]
                        )

    nc.compile()
    return nc


def get_module(nb=NT):
    key = ("mod", nb, GP_COMBINE, BCAST_BIAS)
    if key not in _CACHE:
        _CACHE[key] = build_module(nb)
    return _CACHE[key]


def make_inputs(x, scale_val, bias):
    import ml_dtypes

    bf16 = ml_dtypes.bfloat16
    lutb = np.ascontiguousarray(make_lut(scale_val).astype(bf16))
    eye = np.eye(P, dtype=np.float32)
    identb = np.ascontiguousarray(np.stack([eye, -eye]).astype(bf16))
    bias = np.ascontiguousarray(bias.astype(np.float32))
    base = {"lutb": lutb, "identb": identb}
    if BCAST_BIAS:
        base["bias1"] = bias[None, :]
    else:
        base["biasr"] = np.ascontiguousarray(
            np.broadcast_to(bias[None, :], (P, OUT))
        )
    xb = np.ascontiguousarray(x.astype(bf16)).reshape(N_CORES, NT // 4, 4, P, IN)
    return [dict(base, x=xb[c]) for c in range(N_CORES)]


def kernel(x, proj, scale, bias):
    from concourse.bass_utils import run_bass_kernel_spmd

    x = np.ascontiguousarray(np.asarray(x, dtype=np.float32))
    bias = np.asarray(bias, dtype=np.float32)
    scale_val = float(np.asarray(scale).reshape(-1)[0])
    del proj  # deterministic +-1 Hadamard; regenerated as the sign-pattern LUT

    nc = get_module()
    in_maps = make_inputs(x, scale_val, bias)
    res = run_bass_kernel_spmd(nc, in_maps, core_ids=list(range(N_CORES)))
    return np.concatenate([res.results[c]["out"] for c in range(N_CORES)], axis=0)
